# revision 2
# baseline (speedup 1.0000x reference)
"""Multi-head attention + post-LN Trainium2 kernel (8 NeuronCores, SPMD).

Sharding: data-parallel over (batch, seq-half) -> 8 shards. Each core computes
the full 16-head attention for its 1024 query rows against the full 2048-key
context of its batch (K/V projections duplicated across the 2 cores of a
batch), so no cross-core collectives are needed and the post-LN epilogue is
fully local.

Layout strategy per core:
  - inputs transposed on PE (d_model on partitions) to feed projections
  - qh^T [d,1024] and kh^T [d,2048] hold head dims on partitions: a 128-row
    partition tile holds a head PAIR (2x64), so score matmuls (K=64) run as
    row-tiled pairs filling the 128x128 array
  - scores computed twice, in [q,k] orientation (softmax + attn output) and
    [k,q] orientation (P^T for the attn@V matmul) - recomputing on PE+ACT is
    cheaper than transposing 33M elements
  - softmax row sums come free from the ACT exp accumulator; normalization of
    the context path is applied via a DMA-broadcast reciprocal tile
  - all matmuls run in float32r (15-bit mantissa, full PE rate at N>=512)
"""

import math

import numpy as np

import concourse.bass as bass
import concourse.mybir as mybir
import concourse.tile as tile
from concourse import bacc, bass_utils
from concourse.masks import make_identity

B, S, D = 4, 2048, 1024
H, DK = 16, 64
SQL = S // 2  # per-core query rows
NCORES = 8

F32 = mybir.dt.float32
F32R = mybir.dt.float32r
AF = mybir.ActivationFunctionType
ALU = mybir.AluOpType

_CACHED_NC = None


def _dram_ap(t, offset, ap):
    base = t[:] if len(t.shape) == 1 else t[:, :]
    return bass.AP(tensor=base.tensor, offset=offset, ap=ap)


def build_kernel():
    nc = bacc.Bacc("TRN2")

    xq = nc.dram_tensor("xq", [SQL, D], F32, kind="ExternalInput")
    xk = nc.dram_tensor("xk", [S, D], F32, kind="ExternalInput")
    xv = nc.dram_tensor("xv", [S, D], F32, kind="ExternalInput")
    wq = nc.dram_tensor("wq", [D, D], F32, kind="ExternalInput")
    wk = nc.dram_tensor("wk", [D, D], F32, kind="ExternalInput")
    wv = nc.dram_tensor("wv", [D, D], F32, kind="ExternalInput")
    wo = nc.dram_tensor("wo", [D, D], F32, kind="ExternalInput")
    bq = nc.dram_tensor("bq", [D], F32, kind="ExternalInput")
    bk = nc.dram_tensor("bk", [D], F32, kind="ExternalInput")
    co = nc.dram_tensor("co", [D], F32, kind="ExternalInput")  # b_o + b_v @ w_o.T
    gamma = nc.dram_tensor("gamma", [D], F32, kind="ExternalInput")
    beta = nc.dram_tensor("beta", [D], F32, kind="ExternalInput")

    attn_o = nc.dram_tensor("attn_o", [H, SQL, S], F32, kind="ExternalOutput")
    out_o = nc.dram_tensor("out_o", [SQL, D], F32, kind="ExternalOutput")

    khT_d = nc.dram_tensor("khT_d", [D, S], F32R, kind="Internal")
    ctxT_d = nc.dram_tensor("ctxT_d", [D, SQL], F32R, kind="Internal")
    recips_d = nc.dram_tensor("recips_d", [H * SQL], F32, kind="Internal")

    scale = 1.0 / math.sqrt(DK)

    with tile.TileContext(nc) as tc:
        with tc.tile_pool(name="persist", bufs=1) as pp:
            ident = pp.tile([128, 128], F32)
            make_identity(nc, ident)
            co_bc = pp.tile([128, D], F32)
            gamma_bc = pp.tile([128, D], F32)
            beta_bc = pp.tile([128, D], F32)
            for dst, src in ((co_bc, co), (gamma_bc, gamma), (beta_bc, beta)):
                nc.gpsimd.dma_start(
                    out=dst, in_=_dram_ap(src, 0, [[0, 128], [1, D]])
                )
            eps_t = pp.tile([128, 1], F32)
            nc.vector.memset(eps_t, 1e-5)

            qhT = pp.tile([128, 8, SQL], F32R)  # [d-pair-tile rows, dout_t, q]
            vh = pp.tile([128, 16, D], F32R)  # [s rows, s_tile, d]
            recips = pp.tile([128, H, 8], F32)  # [q rows, head, q_tile]

            # per-partition bias columns: bias[d] -> [128, 8] (col = dout tile)
            bqT = pp.tile([128, 8], F32)
            bkT = pp.tile([128, 8], F32)
            with tc.tile_pool(name="bias_ps", bufs=1, space="PSUM") as bps:
                for dst, src in ((bqT, bq), (bkT, bk)):
                    braw = pp.tile([8, 128], F32, tag="braw")
                    nc.sync.dma_start(
                        out=braw, in_=src[:].rearrange("(t p) -> t p", p=128)
                    )
                    btp = bps.tile([128, 8], F32)
                    nc.tensor.transpose(btp, braw, ident[0:8, 0:8])
                    nc.vector.tensor_copy(dst, btp)

            # ---------------- Phase B: projections ----------------
            with (
                tc.tile_pool(name="projB", bufs=2) as pb,
                tc.tile_pool(name="wTpool", bufs=1) as pw,
                tc.tile_pool(name="psT", bufs=4, space="PSUM") as psT,
                tc.tile_pool(name="psA", bufs=4, space="PSUM") as psA,
            ):
                for which in ("q", "k", "v"):
                    w_in = {"q": wq, "k": wk, "v": wv}[which]
                    x_in = {"q": xq, "k": xk, "v": xv}[which]
                    n_sblk = {"q": SQL // 512, "k": S // 512, "v": S // 512}[which]

                    wT = pw.tile([128, 8, D], F32R, tag="wT")
                    for r in range(8):
                        wn = pb.tile([128, D], F32, tag="wn")
                        nc.sync.dma_start(out=wn, in_=w_in[r * 128 : (r + 1) * 128, :])
                        for i in range(8):
                            tp = psT.tile([128, 128], F32, tag="tp")
                            nc.tensor.transpose(tp, wn[:, i * 128 : (i + 1) * 128], ident)
                            nc.vector.tensor_copy(wT[:, i, r * 128 : (r + 1) * 128], tp)

                    for sb in range(n_sblk):
                        xT = pb.tile([128, 8, 512], F32R, tag="xT")
                        for st in range(4):
                            xn = pb.tile([128, D], F32, tag="xn")
                            nc.sync.dma_start(
                                out=xn, in_=x_in[sb * 512 + st * 128 : sb * 512 + (st + 1) * 128, :]
                            )
                            for i in range(8):
                                tp = psT.tile([128, 128], F32, tag="tp")
                                nc.tensor.transpose(tp, xn[:, i * 128 : (i + 1) * 128], ident)
                                nc.vector.tensor_copy(xT[:, i, st * 128 : (st + 1) * 128], tp)

                        if which in ("q", "k"):
                            for r in range(8):
                                ps = psA.tile([128, 512], F32, tag="ps")
                                for i in range(8):
                                    nc.tensor.matmul(
                                        ps,
                                        wT[:, i, r * 128 : (r + 1) * 128],
                                        xT[:, i, :],
                                        start=(i == 0),
                                        stop=(i == 7),
                                    )
                                if which == "q":
                                    nc.vector.tensor_scalar_add(
                                        qhT[:, r, sb * 512 : (sb + 1) * 512], ps, bqT[:, r : r + 1]
                                    )
                                else:
                                    kst = pb.tile([128, 512], F32R, tag="kst")
                                    nc.vector.tensor_scalar_add(kst, ps, bkT[:, r : r + 1])
                                    nc.sync.dma_start(
                                        out=khT_d[r * 128 : (r + 1) * 128, sb * 512 : (sb + 1) * 512],
                                        in_=kst,
                                    )
                        else:
                            for st in range(4):
                                for db in range(2):
                                    ps = psA.tile([128, 512], F32, tag="ps")
                                    for i in range(8):
                                        nc.tensor.matmul(
                                            ps,
                                            xT[:, i, st * 128 : (st + 1) * 128],
                                            wT[:, i, db * 512 : (db + 1) * 512],
                                            start=(i == 0),
                                            stop=(i == 7),
                                        )
                                    nc.vector.tensor_copy(
                                        vh[:, sb * 4 + st, db * 512 : (db + 1) * 512], ps
                                    )

            # ---------------- Phase C: attention ----------------
            with (
                tc.tile_pool(name="attC", bufs=2) as pc,
                tc.tile_pool(name="etC", bufs=3) as pet,
                tc.tile_pool(name="ps1", bufs=1, space="PSUM") as ps1,
                tc.tile_pool(name="ps2", bufs=1, space="PSUM") as ps2,
                tc.tile_pool(name="psC", bufs=1, space="PSUM") as psC,
            ):
                for hp in range(8):
                    h0, h1 = 2 * hp, 2 * hp + 1
                    khs = pc.tile([128, S], F32R, tag="khs")
                    nc.sync.dma_start(out=khs, in_=khT_d[hp * 128 : (hp + 1) * 128, :])

                    # pass 1: scores [q,k], softmax, attn output
                    for qt in range(8):
                        qsl = slice(qt * 128, (qt + 1) * 128)
                        E0 = pc.tile([128, S], F32, tag="E0")
                        E1 = pc.tile([128, S], F32, tag="E1")
                        acc = pc.tile([128, 2, 2], F32, tag="acc")  # [q, head, half]
                        for half in range(2):
                            s1a = ps1.tile([128, 1024], F32, tag="s1a")
                            s1b = ps1.tile([128, 1024], F32, tag="s1b")
                            for kb in range(2):
                                kcol = slice(half * 1024 + kb * 512, half * 1024 + (kb + 1) * 512)
                                nc.tensor.matmul(
                                    s1a[:, kb * 512 : (kb + 1) * 512],
                                    qhT[0:64, hp, qsl],
                                    khs[0:64, kcol],
                                    start=True,
                                    stop=True,
                                    tile_position=(0, 0),
                                )
                                nc.tensor.matmul(
                                    s1b[:, kb * 512 : (kb + 1) * 512],
                                    qhT[64:128, hp, qsl],
                                    khs[64:128, kcol],
                                    start=True,
                                    stop=True,
                                    tile_position=(64, 0),
                                )
                            hsl = slice(half * 1024, (half + 1) * 1024)
                            nc.scalar.activation(
                                out=E0[:, hsl], in_=s1a, func=AF.Exp, scale=scale,
                                accum_out=acc[:, 0, half : half + 1],
                            )
                            nc.scalar.activation(
                                out=E1[:, hsl], in_=s1b, func=AF.Exp, scale=scale,
                                accum_out=acc[:, 1, half : half + 1],
                            )
                        for hi, (h, E) in enumerate(((h0, E0), (h1, E1))):
                            ssum = pc.tile([128, 1], F32, tag="ssum")
                            nc.vector.tensor_add(ssum, acc[:, hi, 0:1], acc[:, hi, 1:2])
                            nc.vector.reciprocal(recips[:, h, qt : qt + 1], ssum)
                            nc.vector.tensor_scalar_mul(E, E, recips[:, h, qt : qt + 1])
                            nc.sync.dma_start(out=attn_o[h, qsl, :], in_=E)

                    # recips of these 2 heads -> DRAM (q-major per head)
                    for h in (h0, h1):
                        nc.gpsimd.dma_start(
                            out=_dram_ap(recips_d, h * SQL, [[1, 128], [128, 8]]),
                            in_=recips[:, h, :],
                        )

                    # pass 2: scores^T [k,q], exp, ctx accumulation
                    for qb in range(2):
                        qcol = slice(qb * 512, (qb + 1) * 512)
                        rc = pc.tile([128, 512], F32, tag="rc")
                        nc.gpsimd.dma_start(
                            out=rc[0:64, :],
                            in_=_dram_ap(recips_d, h0 * SQL + qb * 512, [[0, 64], [1, 512]]),
                        )
                        nc.gpsimd.dma_start(
                            out=rc[64:128, :],
                            in_=_dram_ap(recips_d, h1 * SQL + qb * 512, [[0, 64], [1, 512]]),
                        )
                        cx0 = psC.tile([64, 512], F32, tag="cx0")
                        cx1 = psC.tile([64, 512], F32, tag="cx1")
                        for kt in range(16):
                            ksl = slice(kt * 128, (kt + 1) * 128)
                            st0 = ps2.tile([128, 512], F32, tag="st0")
                            st1 = ps2.tile([128, 512], F32, tag="st1")
                            nc.tensor.matmul(
                                st0, khs[0:64, ksl], qhT[0:64, hp, qcol],
                                start=True, stop=True, tile_position=(0, 0),
                            )
                            nc.tensor.matmul(
                                st1, khs[64:128, ksl], qhT[64:128, hp, qcol],
                                start=True, stop=True, tile_position=(64, 0),
                            )
                            ET0 = pet.tile([128, 512], F32R, tag="ET0")
                            ET1 = pet.tile([128, 512], F32R, tag="ET1")
                            nc.scalar.activation(out=ET0, in_=st0, func=AF.Exp, scale=scale)
                            nc.scalar.activation(out=ET1, in_=st1, func=AF.Exp, scale=scale)
                            nc.tensor.matmul(
                                cx0, vh[:, kt, h0 * 64 : (h0 + 1) * 64], ET0,
                                start=(kt == 0), stop=(kt == 15),
                            )
                            nc.tensor.matmul(
                                cx1, vh[:, kt, h1 * 64 : (h1 + 1) * 64], ET1,
                                start=(kt == 0), stop=(kt == 15),
                            )
                        cxs = pc.tile([128, 512], F32R, tag="cxs")
                        nc.vector.tensor_tensor(
                            out=cxs[0:64, :], in0=cx0, in1=rc[0:64, :], op=ALU.mult
                        )
                        nc.vector.tensor_tensor(
                            out=cxs[64:128, :], in0=cx1, in1=rc[64:128, :], op=ALU.mult
                        )
                        nc.sync.dma_start(
                            out=ctxT_d[hp * 128 : (hp + 1) * 128, qcol], in_=cxs
                        )

            # ---------------- Phase D: output projection + LN ----------------
            with (
                tc.tile_pool(name="projD", bufs=2) as pd,
                tc.tile_pool(name="woD", bufs=1) as pwo,
                tc.tile_pool(name="psDT", bufs=4, space="PSUM") as psDT,
                tc.tile_pool(name="psD", bufs=4, space="PSUM") as psD,
            ):
                woT = pwo.tile([128, 8, D], F32R, tag="woT")
                for r in range(8):
                    wn = pd.tile([128, D], F32, tag="wn")
                    nc.sync.dma_start(out=wn, in_=wo[r * 128 : (r + 1) * 128, :])
                    for i in range(8):
                        tp = psDT.tile([128, 128], F32, tag="tp")
                        nc.tensor.transpose(tp, wn[:, i * 128 : (i + 1) * 128], ident)
                        nc.vector.tensor_copy(woT[:, i, r * 128 : (r + 1) * 128], tp)

                for sb in range(2):
                    ctxp = pd.tile([128, 8, 512], F32R, tag="ctxp")
                    for i in range(8):
                        nc.sync.dma_start(
                            out=ctxp[:, i, :],
                            in_=ctxT_d[i * 128 : (i + 1) * 128, sb * 512 : (sb + 1) * 512],
                        )
                    for st in range(4):
                        row0 = sb * 512 + st * 128
                        qres = pd.tile([128, D], F32, tag="qres")
                        nc.sync.dma_start(out=qres, in_=xq[row0 : row0 + 128, :])
                        osb = pd.tile([128, D], F32, tag="osb")
                        for db in range(2):
                            ps = psD.tile([128, 512], F32, tag="ps")
                            for i in range(8):
                                nc.tensor.matmul(
                                    ps,
                                    ctxp[:, i, st * 128 : (st + 1) * 128],
                                    woT[:, i, db * 512 : (db + 1) * 512],
                                    start=(i == 0),
                                    stop=(i == 7),
                                )
                            dsl = slice(db * 512, (db + 1) * 512)
                            nc.vector.tensor_tensor(
                                out=osb[:, dsl], in0=ps, in1=qres[:, dsl], op=ALU.add
                            )
                        nc.vector.tensor_tensor(out=osb, in0=osb, in1=co_bc, op=ALU.add)
                        stats = pd.tile([128, 2, 6], F32, tag="stats")
                        nc.vector.bn_stats(out=stats[:, 0, :], in_=osb[:, 0:512])
                        nc.vector.bn_stats(out=stats[:, 1, :], in_=osb[:, 512:1024])
                        mv = pd.tile([128, 2], F32, tag="mv")
                        nc.vector.bn_aggr(out=mv, in_=stats)
                        std = pd.tile([128, 1], F32, tag="std")
                        nc.scalar.activation(
                            out=std, in_=mv[:, 1:2], func=AF.Sqrt, bias=eps_t
                        )
                        rstd = pd.tile([128, 1], F32, tag="rstd")
                        nc.vector.reciprocal(rstd, std)
                        nc.vector.tensor_scalar(
                            out=osb, in0=osb, scalar1=mv[:, 0:1], scalar2=rstd,
                            op0=ALU.subtract, op1=ALU.mult,
                        )
                        nc.vector.tensor_tensor(out=osb, in0=osb, in1=gamma_bc, op=ALU.mult)
                        nc.vector.tensor_tensor(out=osb, in0=osb, in1=beta_bc, op=ALU.add)
                        nc.sync.dma_start(out=out_o[row0 : row0 + 128, :], in_=osb)

    nc.finalize()
    return nc


def get_nc():
    global _CACHED_NC
    if _CACHED_NC is None:
        _CACHED_NC = build_kernel()
    return _CACHED_NC


def kernel(q, k, v, w_q, b_q, w_k, b_k, w_v, b_v, w_o, b_o, ln_gamma, ln_beta):
    q = np.asarray(q, dtype=np.float32)
    k = np.asarray(k, dtype=np.float32)
    v = np.asarray(v, dtype=np.float32)
    w_q = np.asarray(w_q, dtype=np.float32)
    w_k = np.asarray(w_k, dtype=np.float32)
    w_v = np.asarray(w_v, dtype=np.float32)
    w_o = np.asarray(w_o, dtype=np.float32)
    b_q = np.asarray(b_q, dtype=np.float32)
    b_k = np.asarray(b_k, dtype=np.float32)
    b_v = np.asarray(b_v, dtype=np.float32)
    b_o = np.asarray(b_o, dtype=np.float32)
    ln_gamma = np.asarray(ln_gamma, dtype=np.float32)
    ln_beta = np.asarray(ln_beta, dtype=np.float32)

    co = (b_o + b_v @ w_o.T).astype(np.float32)

    nc = get_nc()
    in_maps = []
    for c in range(NCORES):
        b, half = c // 2, c % 2
        in_maps.append(
            {
                "xq": np.ascontiguousarray(q[b, half * SQL : (half + 1) * SQL, :]),
                "xk": np.ascontiguousarray(k[b]),
                "xv": np.ascontiguousarray(v[b]),
                "wq": w_q,
                "wk": w_k,
                "wv": w_v,
                "wo": w_o,
                "bq": b_q,
                "bk": b_k,
                "co": co,
                "gamma": ln_gamma,
                "beta": ln_beta,
            }
        )

    res = bass_utils.run_bass_kernel_spmd(nc, in_maps, core_ids=list(range(NCORES)))

    out = np.empty((B, S, D), dtype=np.float32)
    attn = np.empty((B, H, S, S), dtype=np.float32)
    for c in range(NCORES):
        b, half = c // 2, c % 2
        sl = slice(half * SQL, (half + 1) * SQL)
        out[b, sl, :] = res.results[c]["out_o"]
        attn[b, :, sl, :] = res.results[c]["attn_o"]
    return out, attn
